# revision 1
# baseline (speedup 1.0000x reference)
"""Trainium2 Bass kernel for causal MHA (nn_MHA_18743237280339).

Full-input contract: kernel(**inputs) takes the unsharded numpy inputs and
returns the full [2, 4096, 512] output.

Distribution (8 NeuronCores, SPMD single program):
  - tensor-parallel over (batch, head): core i handles batch b=i//4 and
    heads h0=2*(i%4), h0+1. Projections use host-sliced weight columns, so
    every core runs an identical program on different data.
  - attention is flash-style: scores stay in PSUM/SBUF, softmax denominator
    comes free from a ones-augmented V column (M=65 PV matmul), no
    max-subtraction (logits are tiny for this problem's scale).
  - QK^T/PV matmuls and the causal mask multiply are column-restricted to
    the live (unmasked) query range on diagonal key-chunks; the two heads'
    64-deep QK matmuls sit at base partitions 0/64 so the PE row groups
    execute them concurrently.
  - emission is software-pipelined so no engine starves: K/Q projections
    are emitted two blocks ahead and V projections one block ahead via
    per-group work slots, the softmax-normalization tail of block j is
    emitted inside block j+1, and output-projection/gather pieces are
    spread one per attention group.
  - four intra-batch AllGathers (replica groups [[0-3],[4-7]], bf16) of the
    per-head attention outputs; each core then computes the output
    projection for a 128-column d_out slice of its batch's rows.

Host-side work is limited to slicing/transposing/casting inputs and
reassembling the output.
"""

import math

import numpy as np
import ml_dtypes

import concourse.bass as bass
import concourse.bacc as bacc
import concourse.tile as tile
from concourse import mybir
from concourse.bass_utils import run_bass_kernel_spmd

BF16 = mybir.dt.bfloat16
F32 = mybir.dt.float32
F8 = mybir.dt.float8e4
DR = mybir.MatmulPerfMode.DoubleRow

D, H, B, S, HD = 512, 8, 2, 4096, 64
P = 128
NKT = D // P  # 4 contraction tiles of 128
NSB = S // 512  # 8 blocks of 512 rows
WOC = 128  # output-projection columns per core

_CACHE: dict = {}


def _build_nc(body_reps=1, do_collective=True):
    nc = bacc.Bacc("TRN2", target_bir_lowering=False, debug=False, num_devices=8)

    xT_d = nc.declare_dram_parameter("xT", [D, S], BF16, isOutput=False)
    wq_d = nc.declare_dram_parameter("wqT", [D, P], BF16, isOutput=False)
    wk_d = nc.declare_dram_parameter("wkT", [D, P], BF16, isOutput=False)
    wv_d = nc.declare_dram_parameter("wvT", [D, P], BF16, isOutput=False)
    wo_d = nc.declare_dram_parameter("woT", [D, WOC], BF16, isOutput=False)
    bq_d = nc.declare_dram_parameter("bq", [P, 1], F32, isOutput=False)
    bk_d = nc.declare_dram_parameter("bk", [P, 1], F32, isOutput=False)
    wob_d = nc.declare_dram_parameter("wob", [WOC, 1], F32, isOutput=False)
    mask_d = nc.declare_dram_parameter("masks", [4, P, 512], BF16, isOutput=False)
    out_d = nc.declare_dram_parameter("outT", [WOC, S], F32, isOutput=True)

    with tile.TileContext(nc) as tc:
        for r in range(body_reps):
            _build_body(
                tc, xT_d, wq_d, wk_d, wv_d, wo_d, bq_d, bk_d, wob_d, mask_d, out_d,
                tag=f"r{r}", do_collective=do_collective,
            )

    nc.compile()
    return nc


def _build_body(
    tc, xT_d, wq_d, wk_d, wv_d, wo_d, bq_d, bk_d, wob_d, mask_d, out_d, tag="",
    do_collective=True,
):
    nc = tc.nc
    Exp = mybir.ActivationFunctionType.Exp
    GROUP = 3  # score chunks (of 128 keys) per exp batch

    with (
        tc.tile_pool(name=f"const{tag}", bufs=1) as const,
        tc.tile_pool(name=f"kqv{tag}", bufs=1) as kqv,
        tc.tile_pool(name=f"dram{tag}", bufs=1, space="DRAM") as dram,
        tc.tile_pool(name=f"xp{tag}", bufs=3) as xp,
        tc.tile_pool(name=f"ps{tag}", bufs=2, space="PSUM") as psp,
        tc.tile_pool(name=f"pvp{tag}", bufs=2, space="PSUM") as pvp,
        tc.tile_pool(name=f"ptp{tag}", bufs=4) as ptp,
        tc.tile_pool(name=f"att{tag}", bufs=3) as att,
        tc.tile_pool(name=f"rcp{tag}", bufs=2) as rcp,
        tc.tile_pool(name=f"attg{tag}", bufs=2) as attgp,
        tc.tile_pool(name=f"osb{tag}", bufs=3) as osbp,
    ):
        # ---- first x block + constants, in order of first use ----
        xt0 = xp.tile([P, NKT, 512], BF16, tag="xt", name=f"xt{tag}_0")
        nc.sync.dma_start(xt0[:], xT_d[:, 0:512].rearrange("(c p) s -> p c s", p=P))
        bq_sb = const.tile([P, 1], F32, name=f"bq{tag}")
        nc.scalar.dma_start(bq_sb[:], bq_d[:, :])
        bk_sb = const.tile([P, 1], F32, name=f"bk{tag}")
        nc.scalar.dma_start(bk_sb[:], bk_d[:, :])
        wk_sb = const.tile([P, NKT, P], BF16, name=f"wk{tag}")
        nc.scalar.dma_start(wk_sb[:], wk_d[:, :].rearrange("(c p) m -> p c m", p=P))
        wq_sb = const.tile([P, NKT, P], BF16, name=f"wq{tag}")
        nc.scalar.dma_start(wq_sb[:], wq_d[:, :].rearrange("(c p) m -> p c m", p=P))
        mask_sb = const.tile([P, 4, 512], BF16, name=f"mask{tag}")
        nc.scalar.dma_start(mask_sb[:], mask_d[:, :, :].rearrange("c p q -> p c q"))
        wv_sb = const.tile([P, NKT, P], BF16, name=f"wv{tag}")
        nc.scalar.dma_start(wv_sb[:], wv_d[:, :].rearrange("(c p) m -> p c m", p=P))
        wo_sb = const.tile([P, NKT, WOC], BF16, name=f"wo{tag}")
        nc.gpsimd.dma_start(wo_sb[:], wo_d[:, :].rearrange("(c p) m -> p c m", p=P))
        wob_sb = const.tile([WOC, 1], F32, name=f"wob{tag}")
        nc.gpsimd.dma_start(wob_sb[:], wob_d[:, :])
        ones_sb = const.tile([P, HD], BF16, name=f"ones{tag}")
        nc.vector.memset(ones_sb[:], 1.0)
        # pre-warm the ScalarE exp table while the first DMAs run
        warm_sb = const.tile([1, 2], F32, name=f"warm{tag}")
        nc.vector.memset(warm_sb[:], 0.0)
        nc.scalar.activation(warm_sb[0:1, 1:2], warm_sb[0:1, 0:1], Exp)

        # ---- persistent per-core tensors ----
        KT = kqv.tile([P, S], BF16, name=f"KT{tag}")  # 2 heads stacked (64+64)
        QT = kqv.tile([P, S], BF16, name=f"QT{tag}")
        V0 = kqv.tile([P, S // P, HD + 1], BF16, name=f"V0{tag}")
        V1 = kqv.tile([P, S // P, HD + 1], BF16, name=f"V1{tag}")
        nc.vector.memset(V0[:, :, HD : HD + 1], 1.0)
        nc.vector.memset(V1[:, :, HD : HD + 1], 1.0)

        NQ = 4  # gather granularity: NSB // NQ q-blocks per AllGather
        QW = S // NQ
        cc_in = [
            dram.tile([2, HD, QW], BF16, name=f"cci{h}{tag}") for h in range(NQ)
        ]
        cc_out = [
            dram.tile([H, HD, QW], BF16, name=f"cco{h}{tag}") for h in range(NQ)
        ]

        xts = {0: xt0}

        def proj_kq(j):
            sl = slice(512 * j, 512 * (j + 1))
            if j not in xts:
                xts[j] = xp.tile([P, NKT, 512], BF16, tag="xt", name=f"xt{tag}_{j}")
                nc.sync.dma_start(
                    xts[j][:], xT_d[:, sl].rearrange("(c p) s -> p c s", p=P)
                )
            xt = xts[j]
            kq = psp.tile([P, GROUP * 512], F32, tag="sp", name=f"kq{tag}_{j}")
            for kt in range(NKT):
                nc.tensor.matmul(
                    kq[:, 0:512],
                    lhsT=wk_sb[:, kt, :],
                    rhs=xt[:, kt, :],
                    start=(kt == 0),
                    stop=(kt == NKT - 1),
                )
            nc.vector.tensor_scalar_add(KT[:, sl], kq[:, 0:512], bk_sb[:])
            for kt in range(NKT):
                nc.tensor.matmul(
                    kq[:, 512:1024],
                    lhsT=wq_sb[:, kt, :],
                    rhs=xt[:, kt, :],
                    start=(kt == 0),
                    stop=(kt == NKT - 1),
                )
            nc.vector.tensor_scalar_add(QT[:, sl], kq[:, 512:1024], bq_sb[:])

        def proj_v(j):
            xt = xts.pop(j)
            vps = psp.tile([P, GROUP * 512], F32, tag="sp", name=f"vps{tag}_{j}")
            for sc in range(2):
                for t in range(2):
                    for kt in range(NKT):
                        nc.tensor.matmul(
                            vps[:, 512 * sc + P * t : 512 * sc + P * (t + 1)],
                            lhsT=xt[:, kt, 256 * sc + P * t : 256 * sc + P * (t + 1)],
                            rhs=wv_sb[:, kt, :],
                            start=(kt == 0),
                            stop=(kt == NKT - 1),
                        )
            for sc in range(2):
                ch = 4 * j + 2 * sc
                base = 512 * sc
                # two key-chunks per strided copy: cols {0:64}+{128:192} of
                # the 256-col [v0|v1] pair region
                nc.vector.tensor_copy(
                    V0[:, ch : ch + 2, 0:HD],
                    vps[:, base : base + 256].rearrange("p (c q) -> p c q", q=P)[
                        :, :, 0:HD
                    ],
                )
                nc.vector.tensor_copy(
                    V1[:, ch : ch + 2, 0:HD],
                    vps[:, base + HD : base + HD + 256].rearrange(
                        "p (c q) -> p c q", q=P
                    )[:, :, 0:HD],
                )

        pending = []  # deferred emission thunks (gather pieces), drained
        # one per attention group so PE/DMA work spreads between exp batches

        def attn_block(j, prev_tail):
            qsl = slice(512 * j, 512 * (j + 1))
            nch = 4 * (j + 1)
            nst = (nch + 5) // 6  # supertiles of 6 chunks
            # deferred-work slots, one consumed per attention group
            slots = []
            if j + 2 < NSB:
                slots.append(lambda: proj_kq(j + 2))
            if j + 1 < NSB:
                slots.append(lambda: proj_v(j + 1))
            if prev_tail is not None:
                slots.append(prev_tail)
            pv = []  # allocated lazily at first PV emission (after prev tail)
            pts = [
                [
                    ptp.tile([P, 6, 512], BF16, tag="pt", name=f"pt{tag}_{p}_{j}_{s}")
                    for s in range(nst)
                ]
                for p in range(2)
            ]
            next_pair = [0, 0]

            def emit_pairs(upto):
                if not pv:
                    for p in range(2):
                        pv.append(
                            pvp.tile([P, 512], F32, tag="pv", name=f"pv{tag}_{p}_{j}")
                        )
                for p in range(2):
                    Vp = V0 if p == 0 else V1
                    while next_pair[p] < upto:
                        kc = next_pair[p]
                        q0 = 128 * (kc - 4 * j) if kc >= 4 * j else 0
                        nc.tensor.matmul(
                            pv[p][0 : HD + 1, q0:512],
                            lhsT=Vp[:, kc, 0 : HD + 1],
                            rhs=pts[p][kc // 6][:, kc % 6, q0:512],
                            start=(kc == 0),
                            stop=(kc == nch - 1),
                        )
                        next_pair[p] += 1

            for gi, g0 in enumerate(range(0, nch, GROUP)):
                gs = min(GROUP, nch - g0)
                sp = [
                    psp.tile(
                        [P, GROUP * 512], F32, tag="sp", name=f"sp{tag}_{p}_{j}_{g0}"
                    )
                    for p in range(2)
                ]
                for t in range(gs):
                    kc = g0 + t
                    q0 = 128 * (kc - 4 * j) if kc >= 4 * j else 0
                    for p in range(2):
                        base = HD * p
                        nc.tensor.matmul(
                            sp[p][:, 512 * t + q0 : 512 * (t + 1)],
                            lhsT=KT[base : base + HD, P * kc : P * (kc + 1)],
                            rhs=QT[base : base + HD, 512 * j + q0 : 512 * (j + 1)],
                            start=True,
                            stop=True,
                        )
                st_i, sl_i = g0 // 6, g0 % 6  # supertile / slot of first chunk
                for p in range(2):
                    nc.scalar.activation(
                        pts[p][st_i][:, sl_i : sl_i + gs, :],
                        sp[p][:, 0 : 512 * gs],
                        Exp,
                    )
                for t in range(gs):
                    kc = g0 + t
                    if kc >= 4 * j:
                        m = kc - 4 * j
                        q0 = 128 * m
                        for p in range(2):
                            pslice = pts[p][kc // 6][:, kc % 6, q0:512]
                            nc.vector.tensor_mul(
                                pslice, pslice, mask_sb[:, m, q0:512]
                            )
                # feed other engines while ScalarE chews this group's exps
                if gi >= 1:
                    if slots:
                        slots.pop(0)()
                    elif pending:
                        pending.pop(0)()
                if gi >= 3:
                    emit_pairs(g0 + gs)
            for s in slots:
                s()
            emit_pairs(nch)

            def tail():
                rb = psp.tile([P, GROUP * 512], F32, tag="sp", name=f"rb{tag}_{j}")
                for p in range(2):
                    rc = rcp.tile([P, 512], BF16, tag="rc", name=f"rc{tag}_{p}_{j}")
                    with nc.allow_low_precision(reason="bf16 recip, rel 4e-3"):
                        nc.vector.reciprocal(
                            rc[HD : HD + 1, :], pv[p][HD : HD + 1, :]
                        )
                    nc.tensor.matmul(
                        rb[HD * p : HD * (p + 1), 0:512],
                        lhsT=ones_sb[HD : HD + 1, 0:HD],
                        rhs=rc[HD : HD + 1, :],
                        start=True,
                        stop=True,
                    )
                    rbs = rcp.tile([HD, 512], F32, tag="rbs", name=f"rbs{tag}_{p}_{j}")
                    nc.vector.tensor_copy(rbs[:], rb[HD * p : HD * (p + 1), 0:512])
                    st = att.tile([HD, 512], BF16, tag="st", name=f"st{tag}_{p}_{j}")
                    nc.vector.tensor_mul(st[:], pv[p][0:HD, :], rbs[:])
                    quarter, col = divmod(512 * j, QW)
                    nc.sync.dma_start(cc_in[quarter][p, :, col : col + 512], st[:])

            return tail

        def gather_items(q):
            attg = attgp.tile([P, NKT, QW], BF16, tag="attg", name=f"ag{tag}_{q}")

            def start_gather():
                if do_collective:
                    nc.gpsimd.collective_compute(
                        "AllGather",
                        mybir.AluOpType.bypass,
                        replica_groups=[[0, 1, 2, 3], [4, 5, 6, 7]],
                        ins=[cc_in[q][:].opt()],
                        outs=[cc_out[q][:].opt()],
                    )
                # cc_out[q] viewed as [h, 64, s]: this batch's 8 heads
                for c in range(NKT):
                    nc.sync.dma_start(
                        attg[:, c, :],
                        cc_out[q][2 * c : 2 * c + 2, :, :].rearrange(
                            "h p s -> (h p) s"
                        ),
                    )

            def wo_chunk(jh):
                ssl = slice(512 * jh, 512 * (jh + 1))
                osl = slice(QW * q + 512 * jh, QW * q + 512 * (jh + 1))
                pw = pvp.tile([P, 512], F32, tag="pv", name=f"pw{tag}_{q}_{jh}")
                for c in range(NKT):
                    nc.tensor.matmul(
                        pw[:, 0:512],
                        lhsT=wo_sb[:, c, :],
                        rhs=attg[:, c, ssl],
                        start=(c == 0),
                        stop=(c == NKT - 1),
                    )
                ot = osbp.tile([WOC, 512], F32, tag="ot", name=f"ot{tag}_{q}_{jh}")
                nc.vector.tensor_scalar_add(ot[:], pw[:, 0:512], wob_sb[:])
                nc.sync.dma_start(out_d[:, osl], ot[:])

            return [start_gather] + [
                (lambda jh=jh: wo_chunk(jh)) for jh in range(QW // 512)
            ]

        per_q = NSB // NQ
        proj_kq(0)
        proj_v(0)
        proj_kq(1)
        prev_tail = None
        for j in range(NSB):
            prev_tail = attn_block(j, prev_tail)
            if (j + 1) % per_q == 0:
                q = (j + 1) // per_q - 1
                if j != NSB - 1:
                    pending.extend(gather_items(q))
                else:
                    prev_tail()
                    prev_tail = None
                    for it in pending:
                        it()
                    pending.clear()
                    for it in gather_items(q):
                        it()


def _get_nc():
    if "nc" not in _CACHE:
        _CACHE["nc"] = _build_nc()
    return _CACHE["nc"]


def _prepare_in_maps(x, wq_w, wq_b, wk_w, wk_b, wv_w, wv_b, wo_w, wo_b):
    bf16 = ml_dtypes.bfloat16
    f32 = np.float32
    x = np.asarray(x, f32)
    wq_w = np.asarray(wq_w, f32)
    wq_b = np.asarray(wq_b, f32)
    wk_w = np.asarray(wk_w, f32)
    wk_b = np.asarray(wk_b, f32)
    wv_w = np.asarray(wv_w, f32)
    wv_b = np.asarray(wv_b, f32)
    wo_w = np.asarray(wo_w, f32)
    wo_b = np.asarray(wo_b, f32)

    scale = f32(1.0 / math.sqrt(D))
    wo_b_eff = wo_b + wo_w @ wv_b

    qi = np.arange(512)[None, :]
    ki = np.arange(P)[:, None]
    masks = np.stack(
        [(ki + 128 * c <= qi).astype(f32) for c in range(4)], axis=0
    )  # [4,128,512]
    masks_bf = np.ascontiguousarray(masks.astype(bf16))

    xT = [np.ascontiguousarray(x[b].T).astype(bf16) for b in range(B)]

    in_maps = []
    for i in range(8):
        b = i // 4
        h0 = 2 * (i % 4)
        hs = slice(64 * h0, 64 * h0 + 128)
        cs = slice(WOC * (i % 4), WOC * (i % 4) + WOC)
        in_maps.append(
            {
                "xT": xT[b],
                "wqT": np.ascontiguousarray((wq_w[hs, :] * scale).T).astype(bf16),
                "wkT": np.ascontiguousarray(wk_w[hs, :].T).astype(bf16),
                "wvT": np.ascontiguousarray(wv_w[hs, :].T).astype(bf16),
                "woT": np.ascontiguousarray(wo_w[cs, :].T).astype(bf16),
                "bq": np.ascontiguousarray((wq_b[hs] * scale).reshape(P, 1)),
                "bk": np.ascontiguousarray(wk_b[hs].reshape(P, 1)),
                "wob": np.ascontiguousarray(wo_b_eff[cs].reshape(WOC, 1)),
                "masks": masks_bf,
            }
        )
    return in_maps


def kernel(
    x, wq_w, wq_b, wk_w, wk_b, wv_w, wv_b, wo_w, wo_b, trace=False, **run_kwargs
):
    in_maps = _prepare_in_maps(x, wq_w, wq_b, wk_w, wk_b, wv_w, wv_b, wo_w, wo_b)
    res = run_bass_kernel_spmd(
        _get_nc(), in_maps, core_ids=list(range(8)), trace=trace, **run_kwargs
    )
    _CACHE["last_result"] = res
    out = np.zeros((B, S, D), np.float32)
    for i in range(8):
        oT = res.results[i]["outT"]  # [128, S]
        b = i // 4
        c0 = WOC * (i % 4)
        out[b, :, c0 : c0 + WOC] = oT.T
    return out

